# revision 23
# baseline (speedup 1.0000x reference)
"""Additive-attention Bass kernel for Trainium2, data-parallel over batch on 8 cores.

Math per batch b:
    q = queries[b] @ W_q                      # (H,)
    kp[t, h] = sum_d keys[b, t, d] W_k[d, h]  # (Tk, H)
    feat = tanh(q + kp)                       # (Tk, H)
    s[t] = feat[t] . w_v                      # (Tk,)
    attn = softmax(s)                         # = exp(s) / sum exp(s)  (no max-sub
                                              #   needed: |s| <= ||w_v||_1 ~ 13)
    out[b] = attn @ values[b]                 # (H,)

On-chip layout: features are [h(partitions), t(free)] so ACT applies the q bias
per-partition and the w_v dot is an M=1 matmul per 128-t slice. keys arrive
[t, d] and are transposed to [d, t] in-PE (pass-through transpose matmuls).

TWO batches are interleaved per chunk so the tensor engine always has
independent work while the other batch's chain (transpose -> kT copy [DVE] ->
kp -> tanh [ACT] -> scores -> exp [ACT] -> values matmul) crosses engines.
PE issue order per chunk: T(A) T(B) KP(A) V(A) KP(B) V(B) S(A) S(B), with the
lag-1 values flush V using the previous chunk's exp columns. f32->f16 key
casts run on the otherwise-idle Pool (gpsimd) engine, prefetched one chunk
ahead. Z partials are per-partition DVE row-sums of the exp columns; both
interleaved batches accumulate output rows in one shared PSUM bank ([2, D]).
"""

import os

import numpy as np

import concourse.bass as bass
import concourse.mybir as mybir
import concourse.tile as tile
from concourse import bacc
from concourse.bass import ts
from concourse.bass_utils import run_bass_kernel_spmd
from concourse.masks import make_identity

B, TK, D, H = 32, 8192, 256, 256
NCORES = 8
BL = B // NCORES          # batches per core
CHUNK = 512               # t-chunk per compute iteration
NCHUNK = TK // CHUNK
NSUB = CHUNK // 128
TT = 2048                 # t-span per DMA load (16 KB contiguous per partition)
NL = TK // TT             # loads per batch
NCC = TT // CHUNK         # compute chunks per load
NNT = TT // 128           # n-slices per load tile

F32 = mybir.dt.float32
F32R = mybir.dt.float32r
F16 = mybir.dt.float16
AF = mybir.ActivationFunctionType


KDBG = os.environ.get("KDBG") == "1"


def build():
    nc = bacc.Bacc("TRN2", target_bir_lowering=False, debug=False, num_devices=NCORES)
    keys_d = nc.dram_tensor("keys", [BL, TK, D], F32, kind="ExternalInput").ap()
    vals_d = nc.dram_tensor("values", [BL, TK, D], F32R, kind="ExternalInput").ap()
    qrs_d = nc.dram_tensor("queries", [BL, D], F32, kind="ExternalInput").ap()
    wq_d = nc.dram_tensor("W_q", [D, H], F32, kind="ExternalInput").ap()
    wk_d = nc.dram_tensor("W_k", [D, H], F32, kind="ExternalInput").ap()
    wv_d = nc.dram_tensor("w_v", [1, H], F32, kind="ExternalInput").ap()
    out_d = nc.dram_tensor("out", [BL, D], F32, kind="ExternalOutput").ap()
    if KDBG:
        dbg_ec = nc.dram_tensor(
            "dbg_ec", [2, NCHUNK, 128, NSUB], F32R, kind="ExternalOutput"
        ).ap()
        dbg_kt = nc.dram_tensor(
            "dbg_kt", [2, NCHUNK, 128, 2, CHUNK], F16, kind="ExternalOutput"
        ).ap()

    with tile.TileContext(nc) as tc:
        with (
            tc.tile_pool(name="consts", bufs=1) as consts,
            tc.tile_pool(name="kin", bufs=2) as kin,
            tc.tile_pool(name="vin", bufs=2) as vin,
            tc.tile_pool(name="mid", bufs=2) as mid,
            tc.tile_pool(name="small", bufs=2) as small,
        ):
            ident_f32 = consts.tile([128, 128], F32)
            make_identity(nc, ident_f32)
            ident = consts.tile([128, 128], F16)
            nc.vector.tensor_copy(out=ident, in_=ident_f32)
            one11 = consts.tile([1, 1], F32)
            nc.vector.memset(one11, 1.0)
            ones_col = consts.tile([128, 1], F32)
            nc.vector.memset(ones_col, 1.0)
            negc = consts.tile([128, 1], F32)
            nc.vector.memset(negc, -6.0)

            wk_f32 = consts.tile([128, 2, H], F32)
            nc.sync.dma_start(out=wk_f32, in_=wk_d.rearrange("(dt p) h -> p dt h", p=128))
            wk_s = consts.tile([128, 2, H], F16)
            nc.vector.tensor_copy(out=wk_s, in_=wk_f32)
            wq_s = consts.tile([128, 2, H], F32)
            nc.sync.dma_start(out=wq_s, in_=wq_d.rearrange("(dt p) h -> p dt h", p=128))
            wv_row = consts.tile([1, H], F32)
            nc.sync.dma_start(out=wv_row, in_=wv_d)
            q_rows = consts.tile([1, BL * D], F32)
            nc.sync.dma_start(
                out=q_rows, in_=qrs_d.rearrange("b d -> (b d)").rearrange("(o f) -> o f", o=1)
            )

            wv_cols = consts.tile([128, 2], F16)      # w_v as [h, htile] columns
            q_cols = consts.tile([128, BL, 2], F32)  # q biases [h, b, htile]

            # ---- setup: w_v columns and per-batch q biases (all tiny) ----
            with tc.tile_pool(name="setup_ps", bufs=1, space="PSUM") as setup_ps:
                ps_wv = setup_ps.tile([128, 2], F32)
                for ht in range(2):
                    nc.tensor.matmul(
                        out=ps_wv[:, ht : ht + 1],
                        lhsT=wv_row[0:1, ts(ht, 128)],
                        rhs=one11,
                        is_transpose=True,
                    )
                nc.vector.tensor_copy(out=wv_cols, in_=ps_wv)

                for b in range(BL):
                    ps_qc = setup_ps.tile([128, 2], F32, tag="ps_qc")
                    for dt in range(2):
                        nc.tensor.matmul(
                            out=ps_qc[:, dt : dt + 1],
                            lhsT=q_rows[0:1, b * D + dt * 128 : b * D + (dt + 1) * 128],
                            rhs=one11,
                            is_transpose=True,
                        )
                    qc_s = small.tile([128, 2], F32, tag="qc_s")
                    nc.vector.tensor_copy(out=qc_s, in_=ps_qc)
                    ps_q = setup_ps.tile([128, 2], F32, tag="ps_q")
                    for ht in range(2):
                        for dt in range(2):
                            nc.tensor.matmul(
                                out=ps_q[:, ht : ht + 1],
                                lhsT=wq_s[:, dt, ts(ht, 128)],
                                rhs=qc_s[:, dt : dt + 1],
                                start=(dt == 0),
                                stop=(dt == 1),
                            )
                    nc.vector.tensor_copy(out=q_cols[:, b, :], in_=ps_q)

            # ---- main loop: two batches (parities) interleaved per pair ----
            with (
                tc.tile_pool(name="ptr", bufs=1, space="PSUM") as ptrp,
                tc.tile_pool(name="pkp", bufs=2, space="PSUM") as pkpp,
                tc.tile_pool(name="scol", bufs=2, space="PSUM") as scolp,
                tc.tile_pool(name="pout", bufs=1, space="PSUM") as poutp,
            ):
                for pair in range(BL // 2):
                    bs = (2 * pair, 2 * pair + 1)
                    # separate PSUM bank per parity: a group-start marks the
                    # whole 2KB bank pending-zero, so two interleaved
                    # accumulation groups must not share a bank
                    psum_outs = [
                        poutp.tile([1, D], F32, tag=f"po{par}", name=f"po{par}")
                        for par in range(2)
                    ]
                    z_pps = [
                        small.tile([128, NCHUNK], F32, tag=f"zpp{par}", name=f"zpp{par}")
                        for par in range(2)
                    ]
                    pends = [[], []]

                    def flush_pend(par, last):
                        ec_p, vals_p, c_p = pends[par].pop(0)
                        cc_p = c_p % NCC
                        for j in range(NSUB):
                            nc.tensor.matmul(
                                out=psum_outs[par],
                                lhsT=ec_p[:, j : j + 1],
                                rhs=vals_p[:, cc_p * NSUB + j, :],
                                start=(c_p == 0 and j == 0),
                                stop=(last and j == NSUB - 1),
                                skip_group_check=True,
                            )

                    keys_f32s = [None, None]
                    vals_alls = [None, None]
                    keys16s = [None, None]

                    def issue_dma(L):
                        # keys first (needed by cast/transpose at chunk cc=0);
                        # values aren't read until the lag-1 flush a chunk later
                        for par in range(2):
                            b = bs[par]
                            kf = kin.tile([128, NNT, D], F32, tag=f"keys{par}")
                            nc.sync.dma_start(
                                out=kf,
                                in_=keys_d[b, L * TT : (L + 1) * TT, :].rearrange(
                                    "(p n) d -> p n d", p=128
                                ),
                            )
                            keys_f32s[par] = kf
                        for par in range(2):
                            b = bs[par]
                            va = vin.tile([128, NNT, D], F32R, tag=f"vals{par}")
                            nc.sync.dma_start(
                                out=va,
                                in_=vals_d[b, L * TT : (L + 1) * TT, :].rearrange(
                                    "(p n) d -> p n d", p=128
                                ),
                            )
                            vals_alls[par] = va

                    kf_hist = {}  # (L, par) -> keys_f32 tile

                    def issue_cast(L, cc, par):
                        # f32 -> f16 key cast on the (otherwise idle) Pool engine
                        k16 = kin.tile([128, NSUB, D], F16, tag=f"k16_{par}", bufs=2)
                        nc.gpsimd.tensor_copy(
                            out=k16,
                            in_=kf_hist[(L, par)][:, cc * NSUB : (cc + 1) * NSUB, :],
                        )
                        return k16

                    issue_dma(0)
                    kf_hist[(0, 0)] = keys_f32s[0]
                    kf_hist[(0, 1)] = keys_f32s[1]
                    for par in range(2):
                        keys16s[par] = issue_cast(0, 0, par)

                    for L in range(NL):
                        # capture THIS L's values tiles before the prefetch
                        # below overwrites vals_alls with L+1's tiles
                        vals_cur = [vals_alls[0], vals_alls[1]]
                        if L + 1 < NL:
                            issue_dma(L + 1)
                            kf_hist[(L + 1, 0)] = keys_f32s[0]
                            kf_hist[(L + 1, 1)] = keys_f32s[1]
                        for cc in range(NCC):
                            c = L * NCC + cc

                            # ---- T: keys [t, d] -> [d, t] via PE pass-through
                            # transposes, dt-major so each kT half copies early
                            kTs = []
                            for par in range(2):
                                ptr_t = ptrp.tile(
                                    [128, 2, NSUB, 128], F16, tag=f"ptr{par}"
                                )
                                kT = mid.tile([128, 2, CHUNK], F16, tag=f"kT{par}")
                                for dt in range(2):
                                    for j in range(NSUB):
                                        nc.tensor.matmul(
                                            out=ptr_t[:, dt, j, :],
                                            lhsT=keys16s[par][:, j, ts(dt, 128)],
                                            rhs=ident,
                                            is_transpose=True,
                                        )
                                    nc.vector.tensor_copy(
                                        out=kT[:, dt, :], in_=ptr_t[:, dt, :, :]
                                    )
                                kTs.append(kT)

                            # ---- per parity: KP (+tanh) then lag-1 values flush
                            feats = []
                            for par in range(2):
                                kps = []
                                for ht in range(2):
                                    kp = pkpp.tile([128, CHUNK], F32, tag="kp")
                                    kps.append(kp)
                                    for dt in range(2):
                                        nc.tensor.matmul(
                                            out=kp,
                                            lhsT=wk_s[:, dt, ts(ht, 128)],
                                            rhs=kTs[par][:, dt, :],
                                            start=(dt == 0),
                                            stop=(dt == 1),
                                        )
                                feat = mid.tile([128, 2, CHUNK], F16, tag=f"feat{par}")
                                for ht in range(2):
                                    nc.scalar.activation(
                                        out=feat[:, ht, :],
                                        in_=kps[ht],
                                        func=AF.Tanh,
                                        bias=q_cols[:, bs[par], ht : ht + 1],
                                        scale=1.0,
                                    )
                                feats.append(feat)
                                if pends[par]:
                                    flush_pend(par, last=False)

                            # ---- S: score columns, exp, Z partials
                            for par in range(2):
                                scol = scolp.tile([128, NSUB], F32, tag="scol")
                                for j in range(NSUB):
                                    for ht in range(2):
                                        nc.tensor.matmul(
                                            out=scol[:, j : j + 1],
                                            lhsT=feats[par][:, ht, ts(j, 128)],
                                            rhs=wv_cols[:, ht : ht + 1],
                                            start=(ht == 0),
                                            stop=(ht == 1),
                                        )
                                ec = small.tile([128, NSUB], F32R, tag=f"ec{par}", bufs=3)
                                nc.scalar.activation(
                                    out=ec,
                                    in_=scol,
                                    func=AF.Exp,
                                    bias=negc[:, 0:1],
                                )
                                nc.vector.reduce_sum(
                                    out=z_pps[par][:, c : c + 1],
                                    in_=ec,
                                    axis=mybir.AxisListType.X,
                                )
                                if KDBG and pair == 0:
                                    nc.sync.dma_start(out=dbg_ec[par, c], in_=ec)
                                    nc.sync.dma_start(
                                        out=dbg_kt[par, c], in_=kTs[par]
                                    )
                                pends[par].append((ec, vals_cur[par], c))

                            # ---- prefetch next chunk's key casts (Pool engine)
                            if cc + 1 < NCC:
                                for par in range(2):
                                    keys16s[par] = issue_cast(L, cc + 1, par)
                            elif L + 1 < NL:
                                for par in range(2):
                                    keys16s[par] = issue_cast(L + 1, 0, par)

                    # ---- tail: flush last chunk, normalize, store ----
                    for par in range(2):
                        flush_pend(par, last=True)
                    for par in range(2):
                        b = bs[par]
                        # Z = sum over partitions and chunks of z_pp:
                        #   [128,16] x ones -> [16,1] -> transpose -> [1,16] -> sum
                        zt_ps = scolp.tile([16, 1], F32, tag="scol", name=f"zt{par}")
                        nc.tensor.matmul(out=zt_ps, lhsT=z_pps[par], rhs=ones_col)
                        zt_s = small.tile([16, 1], F32, tag=f"zt_s{par}")
                        nc.vector.tensor_copy(out=zt_s, in_=zt_ps)
                        zrow_ps = scolp.tile([1, 16], F32, tag="scol", name=f"zr{par}")
                        nc.tensor.matmul(
                            out=zrow_ps,
                            lhsT=zt_s,
                            rhs=ident_f32[0:16, 0:16],
                            is_transpose=True,
                        )
                        z1 = small.tile([1, 1], F32, tag=f"z{par}")
                        nc.vector.reduce_sum(
                            out=z1, in_=zrow_ps, axis=mybir.AxisListType.X
                        )
                        rz = small.tile([1, 1], F32, tag=f"rz{par}")
                        nc.vector.reciprocal(out=rz, in_=z1)
                        orow = small.tile([1, D], F32, tag=f"orow{par}")
                        nc.scalar.mul(
                            out=orow, in_=psum_outs[par], mul=rz[0:1, 0:1]
                        )
                        nc.sync.dma_start(out=out_d[b : b + 1, :], in_=orow)

    nc.compile()
    return nc


_NC = None


def _get_nc():
    global _NC
    if _NC is None:
        _NC = build()
    return _NC


def kernel(queries, keys, values, W_q, W_k, w_v):
    nc = _get_nc()
    queries = np.asarray(queries, np.float32)
    keys = np.asarray(keys, np.float32)
    values = np.asarray(values, np.float32)
    W_q = np.ascontiguousarray(np.asarray(W_q, np.float32))
    W_k = np.ascontiguousarray(np.asarray(W_k, np.float32))
    wv2 = np.ascontiguousarray(np.asarray(w_v, np.float32).reshape(1, H))
    in_maps = []
    for i in range(NCORES):
        sl = slice(i * BL, (i + 1) * BL)
        in_maps.append(
            {
                "queries": np.ascontiguousarray(queries[sl]),
                "keys": np.ascontiguousarray(keys[sl]),
                "values": np.ascontiguousarray(values[sl]),
                "W_q": W_q,
                "W_k": W_k,
                "w_v": wv2,
            }
        )
    res = run_bass_kernel_spmd(nc, in_maps, list(range(NCORES)))
    return np.concatenate([res.results[i]["out"] for i in range(NCORES)], axis=0)


# revision 24
# speedup vs baseline: 1.2263x; 1.2263x over previous
"""Additive-attention Bass kernel for Trainium2, data-parallel over batch on 8 cores.

Math per batch b:
    q = queries[b] @ W_q                      # (H,)
    kp[t, h] = sum_d keys[b, t, d] W_k[d, h]  # (Tk, H)
    feat = tanh(q + kp)                       # (Tk, H)
    s[t] = feat[t] . w_v                      # (Tk,)
    attn = softmax(s)                         # = exp(s) / sum exp(s)  (no max-sub
                                              #   needed: |s| <= ||w_v||_1 ~ 13)
    out[b] = attn @ values[b]                 # (H,)

On-chip layout: features are [h(partitions), t(free)] so ACT applies the q bias
per-partition and the w_v dot is an M=1 matmul per 128-t slice. keys arrive
[t, d] and are transposed to [d, t] in-PE (pass-through transpose matmuls).

TWO batches are interleaved per chunk so the tensor engine always has
independent work while the other batch's chain (transpose -> kT copy [DVE] ->
kp -> tanh [ACT] -> scores -> exp [ACT] -> values matmul) crosses engines.
PE issue order per chunk: T(A) T(B) KP(A) V(A) KP(B) V(B) S(A) S(B), with the
lag-1 values flush V using the previous chunk's exp columns. f32->f16 key
casts run on the otherwise-idle Pool (gpsimd) engine, prefetched one chunk
ahead. Z partials are per-partition DVE row-sums of the exp columns; both
interleaved batches accumulate output rows in one shared PSUM bank ([2, D]).
"""

import os

import numpy as np

import concourse.bass as bass
import concourse.mybir as mybir
import concourse.tile as tile
from concourse import bacc
from concourse.bass import ts
from concourse.bass_utils import run_bass_kernel_spmd
from concourse.masks import make_identity

B, TK, D, H = 32, 8192, 256, 256
NCORES = 8
BL = B // NCORES          # batches per core
CHUNK = 512               # t-chunk per compute iteration
NCHUNK = TK // CHUNK
NSUB = CHUNK // 128
TT = 2048                 # t-span per DMA load (16 KB contiguous per partition)
NL = TK // TT             # loads per batch
NCC = TT // CHUNK         # compute chunks per load
NNT = TT // 128           # n-slices per load tile

F32 = mybir.dt.float32
F32R = mybir.dt.float32r
F16 = mybir.dt.float16
AF = mybir.ActivationFunctionType


KDBG = os.environ.get("KDBG") == "1"


def build():
    nc = bacc.Bacc("TRN2", target_bir_lowering=False, debug=False, num_devices=NCORES)
    keys_d = nc.dram_tensor("keys", [BL, TK, D], F32, kind="ExternalInput").ap()
    vals_d = nc.dram_tensor("values", [BL, TK, D], F32R, kind="ExternalInput").ap()
    qrs_d = nc.dram_tensor("queries", [BL, D], F32, kind="ExternalInput").ap()
    wq_d = nc.dram_tensor("W_q", [D, H], F32, kind="ExternalInput").ap()
    wk_d = nc.dram_tensor("W_k", [D, H], F32, kind="ExternalInput").ap()
    wv_d = nc.dram_tensor("w_v", [1, H], F32, kind="ExternalInput").ap()
    out_d = nc.dram_tensor("out", [BL, D], F32, kind="ExternalOutput").ap()
    if KDBG:
        dbg_ec = nc.dram_tensor(
            "dbg_ec", [2, NCHUNK, 128, NSUB], F32R, kind="ExternalOutput"
        ).ap()
        dbg_kt = nc.dram_tensor(
            "dbg_kt", [2, NCHUNK, 128, 2, CHUNK], F16, kind="ExternalOutput"
        ).ap()

    with tile.TileContext(nc) as tc:
        with (
            tc.tile_pool(name="consts", bufs=1) as consts,
            tc.tile_pool(name="kin", bufs=2) as kin,
            tc.tile_pool(name="vin", bufs=2) as vin,
            tc.tile_pool(name="mid", bufs=2) as mid,
            tc.tile_pool(name="small", bufs=2) as small,
        ):
            ident_f32 = consts.tile([128, 128], F32)
            make_identity(nc, ident_f32)
            ident = consts.tile([128, 128], F16)
            nc.vector.tensor_copy(out=ident, in_=ident_f32)
            one11 = consts.tile([1, 1], F32)
            nc.vector.memset(one11, 1.0)
            ones_col = consts.tile([128, 1], F32)
            nc.vector.memset(ones_col, 1.0)
            negc = consts.tile([128, 1], F32)
            nc.vector.memset(negc, -6.0)

            wk_f32 = consts.tile([128, 2, H], F32)
            nc.sync.dma_start(out=wk_f32, in_=wk_d.rearrange("(dt p) h -> p dt h", p=128))
            wk_s = consts.tile([128, 2, H], F16)
            nc.vector.tensor_copy(out=wk_s, in_=wk_f32)
            wq_s = consts.tile([128, 2, H], F32)
            nc.sync.dma_start(out=wq_s, in_=wq_d.rearrange("(dt p) h -> p dt h", p=128))
            wv_row = consts.tile([1, H], F32)
            nc.sync.dma_start(out=wv_row, in_=wv_d)
            q_rows = consts.tile([1, BL * D], F32)
            nc.sync.dma_start(
                out=q_rows, in_=qrs_d.rearrange("b d -> (b d)").rearrange("(o f) -> o f", o=1)
            )

            wv_cols = consts.tile([128, 2], F16)      # w_v as [h, htile] columns
            q_cols = consts.tile([128, BL, 2], F32)  # q biases [h, b, htile]

            # ---- setup: w_v columns and per-batch q biases (all tiny) ----
            with tc.tile_pool(name="setup_ps", bufs=1, space="PSUM") as setup_ps:
                ps_wv = setup_ps.tile([128, 2], F32)
                for ht in range(2):
                    nc.tensor.matmul(
                        out=ps_wv[:, ht : ht + 1],
                        lhsT=wv_row[0:1, ts(ht, 128)],
                        rhs=one11,
                        is_transpose=True,
                    )
                nc.vector.tensor_copy(out=wv_cols, in_=ps_wv)

                for b in range(BL):
                    ps_qc = setup_ps.tile([128, 2], F32, tag="ps_qc")
                    for dt in range(2):
                        nc.tensor.matmul(
                            out=ps_qc[:, dt : dt + 1],
                            lhsT=q_rows[0:1, b * D + dt * 128 : b * D + (dt + 1) * 128],
                            rhs=one11,
                            is_transpose=True,
                        )
                    qc_s = small.tile([128, 2], F32, tag="qc_s")
                    nc.vector.tensor_copy(out=qc_s, in_=ps_qc)
                    ps_q = setup_ps.tile([128, 2], F32, tag="ps_q")
                    for ht in range(2):
                        for dt in range(2):
                            nc.tensor.matmul(
                                out=ps_q[:, ht : ht + 1],
                                lhsT=wq_s[:, dt, ts(ht, 128)],
                                rhs=qc_s[:, dt : dt + 1],
                                start=(dt == 0),
                                stop=(dt == 1),
                            )
                    nc.vector.tensor_copy(out=q_cols[:, b, :], in_=ps_q)

            # ---- main loop: two batches (parities) interleaved per pair ----
            with (
                tc.tile_pool(name="ptr", bufs=1, space="PSUM") as ptrp,
                tc.tile_pool(name="pkp", bufs=2, space="PSUM") as pkpp,
                tc.tile_pool(name="scol", bufs=2, space="PSUM") as scolp,
                tc.tile_pool(name="pout", bufs=1, space="PSUM") as poutp,
            ):
                for pair in range(BL // 2):
                    bs = (2 * pair, 2 * pair + 1)
                    # separate PSUM bank per parity: a group-start marks the
                    # whole 2KB bank pending-zero, so two interleaved
                    # accumulation groups must not share a bank
                    psum_outs = [
                        poutp.tile([1, D], F32, tag=f"po{par}", name=f"po{par}")
                        for par in range(2)
                    ]
                    z_pps = [
                        small.tile([128, NCHUNK], F32, tag=f"zpp{par}", name=f"zpp{par}")
                        for par in range(2)
                    ]
                    pends = [[], []]

                    def flush_pend(par, last):
                        ec_p, vals_p, c_p = pends[par].pop(0)
                        cc_p = c_p % NCC
                        for j in range(NSUB):
                            nc.tensor.matmul(
                                out=psum_outs[par],
                                lhsT=ec_p[:, j : j + 1],
                                rhs=vals_p[:, cc_p * NSUB + j, :],
                                start=(c_p == 0 and j == 0),
                                stop=(last and j == NSUB - 1),
                                skip_group_check=True,
                            )

                    keys_f32s = [None, None]
                    vals_alls = [None, None]
                    keys16s = [None, None]

                    def issue_dma(L):
                        # keys first (needed by cast/transpose at chunk cc=0);
                        # values aren't read until the lag-1 flush a chunk later
                        for par in range(2):
                            b = bs[par]
                            kf = kin.tile([128, NNT, D], F32, tag=f"keys{par}")
                            nc.sync.dma_start(
                                out=kf,
                                in_=keys_d[b, L * TT : (L + 1) * TT, :].rearrange(
                                    "(p n) d -> p n d", p=128
                                ),
                            )
                            keys_f32s[par] = kf
                        for par in range(2):
                            b = bs[par]
                            va = vin.tile([128, NNT, D], F32R, tag=f"vals{par}")
                            nc.sync.dma_start(
                                out=va,
                                in_=vals_d[b, L * TT : (L + 1) * TT, :].rearrange(
                                    "(p n) d -> p n d", p=128
                                ),
                            )
                            vals_alls[par] = va

                    kf_hist = {}  # (L, par) -> keys_f32 tile

                    def issue_cast(L, cc, par):
                        # f32 -> f16 key cast on DVE (gpsimd is ~5x slower here)
                        k16 = kin.tile([128, NSUB, D], F16, tag=f"k16_{par}", bufs=2)
                        nc.vector.tensor_copy(
                            out=k16,
                            in_=kf_hist[(L, par)][:, cc * NSUB : (cc + 1) * NSUB, :],
                        )
                        return k16

                    issue_dma(0)
                    kf_hist[(0, 0)] = keys_f32s[0]
                    kf_hist[(0, 1)] = keys_f32s[1]
                    for par in range(2):
                        keys16s[par] = issue_cast(0, 0, par)

                    for L in range(NL):
                        # capture THIS L's values tiles before the prefetch
                        # below overwrites vals_alls with L+1's tiles
                        vals_cur = [vals_alls[0], vals_alls[1]]
                        if L + 1 < NL:
                            issue_dma(L + 1)
                            kf_hist[(L + 1, 0)] = keys_f32s[0]
                            kf_hist[(L + 1, 1)] = keys_f32s[1]
                        for cc in range(NCC):
                            c = L * NCC + cc

                            # ---- T: keys [t, d] -> [d, t] via PE pass-through
                            # transposes, dt-major so each kT half copies early
                            kTs = []
                            for par in range(2):
                                ptr_t = ptrp.tile(
                                    [128, 2, NSUB, 128], F16, tag=f"ptr{par}"
                                )
                                kT = mid.tile([128, 2, CHUNK], F16, tag=f"kT{par}")
                                for dt in range(2):
                                    for j in range(NSUB):
                                        nc.tensor.matmul(
                                            out=ptr_t[:, dt, j, :],
                                            lhsT=keys16s[par][:, j, ts(dt, 128)],
                                            rhs=ident,
                                            is_transpose=True,
                                        )
                                    nc.vector.tensor_copy(
                                        out=kT[:, dt, :], in_=ptr_t[:, dt, :, :]
                                    )
                                kTs.append(kT)

                            # ---- per parity: KP (+tanh) then lag-1 values flush
                            feats = []
                            for par in range(2):
                                kps = []
                                for ht in range(2):
                                    kp = pkpp.tile([128, CHUNK], F32, tag="kp")
                                    kps.append(kp)
                                    for dt in range(2):
                                        nc.tensor.matmul(
                                            out=kp,
                                            lhsT=wk_s[:, dt, ts(ht, 128)],
                                            rhs=kTs[par][:, dt, :],
                                            start=(dt == 0),
                                            stop=(dt == 1),
                                        )
                                feat = mid.tile([128, 2, CHUNK], F16, tag=f"feat{par}")
                                for ht in range(2):
                                    nc.scalar.activation(
                                        out=feat[:, ht, :],
                                        in_=kps[ht],
                                        func=AF.Tanh,
                                        bias=q_cols[:, bs[par], ht : ht + 1],
                                        scale=1.0,
                                    )
                                feats.append(feat)
                                if pends[par]:
                                    flush_pend(par, last=False)

                            # ---- S: score columns, exp, Z partials
                            for par in range(2):
                                scol = scolp.tile([128, NSUB], F32, tag="scol")
                                for j in range(NSUB):
                                    for ht in range(2):
                                        nc.tensor.matmul(
                                            out=scol[:, j : j + 1],
                                            lhsT=feats[par][:, ht, ts(j, 128)],
                                            rhs=wv_cols[:, ht : ht + 1],
                                            start=(ht == 0),
                                            stop=(ht == 1),
                                        )
                                ec = small.tile([128, NSUB], F32R, tag=f"ec{par}", bufs=3)
                                nc.scalar.activation(
                                    out=ec,
                                    in_=scol,
                                    func=AF.Exp,
                                    bias=negc[:, 0:1],
                                )
                                nc.vector.reduce_sum(
                                    out=z_pps[par][:, c : c + 1],
                                    in_=ec,
                                    axis=mybir.AxisListType.X,
                                )
                                if KDBG and pair == 0:
                                    nc.sync.dma_start(out=dbg_ec[par, c], in_=ec)
                                    nc.sync.dma_start(
                                        out=dbg_kt[par, c], in_=kTs[par]
                                    )
                                pends[par].append((ec, vals_cur[par], c))

                            # ---- prefetch next chunk's key casts (Pool engine)
                            if cc + 1 < NCC:
                                for par in range(2):
                                    keys16s[par] = issue_cast(L, cc + 1, par)
                            elif L + 1 < NL:
                                for par in range(2):
                                    keys16s[par] = issue_cast(L + 1, 0, par)

                    # ---- tail: flush last chunk, normalize, store ----
                    for par in range(2):
                        flush_pend(par, last=True)
                    for par in range(2):
                        b = bs[par]
                        # Z = sum over partitions and chunks of z_pp:
                        #   [128,16] x ones -> [16,1] -> transpose -> [1,16] -> sum
                        zt_ps = scolp.tile([16, 1], F32, tag="scol", name=f"zt{par}")
                        nc.tensor.matmul(out=zt_ps, lhsT=z_pps[par], rhs=ones_col)
                        zt_s = small.tile([16, 1], F32, tag=f"zt_s{par}")
                        nc.vector.tensor_copy(out=zt_s, in_=zt_ps)
                        zrow_ps = scolp.tile([1, 16], F32, tag="scol", name=f"zr{par}")
                        nc.tensor.matmul(
                            out=zrow_ps,
                            lhsT=zt_s,
                            rhs=ident_f32[0:16, 0:16],
                            is_transpose=True,
                        )
                        z1 = small.tile([1, 1], F32, tag=f"z{par}")
                        nc.vector.reduce_sum(
                            out=z1, in_=zrow_ps, axis=mybir.AxisListType.X
                        )
                        rz = small.tile([1, 1], F32, tag=f"rz{par}")
                        nc.vector.reciprocal(out=rz, in_=z1)
                        orow = small.tile([1, D], F32, tag=f"orow{par}")
                        nc.scalar.mul(
                            out=orow, in_=psum_outs[par], mul=rz[0:1, 0:1]
                        )
                        nc.sync.dma_start(out=out_d[b : b + 1, :], in_=orow)

    nc.compile()
    return nc


_NC = None


def _get_nc():
    global _NC
    if _NC is None:
        _NC = build()
    return _NC


def kernel(queries, keys, values, W_q, W_k, w_v):
    nc = _get_nc()
    queries = np.asarray(queries, np.float32)
    keys = np.asarray(keys, np.float32)
    values = np.asarray(values, np.float32)
    W_q = np.ascontiguousarray(np.asarray(W_q, np.float32))
    W_k = np.ascontiguousarray(np.asarray(W_k, np.float32))
    wv2 = np.ascontiguousarray(np.asarray(w_v, np.float32).reshape(1, H))
    in_maps = []
    for i in range(NCORES):
        sl = slice(i * BL, (i + 1) * BL)
        in_maps.append(
            {
                "queries": np.ascontiguousarray(queries[sl]),
                "keys": np.ascontiguousarray(keys[sl]),
                "values": np.ascontiguousarray(values[sl]),
                "W_q": W_q,
                "W_k": W_k,
                "w_v": wv2,
            }
        )
    res = run_bass_kernel_spmd(nc, in_maps, list(range(NCORES)))
    return np.concatenate([res.results[i]["out"] for i in range(NCORES)], axis=0)
